# revision 2
# baseline (speedup 1.0000x reference)
"""CategorySpecificLinear on 8 TRN2 NeuronCores — v2.

out[b, t, h] = sum_i x[b, t, i] * W[cat_ids[b], i, h] + bias[cat_ids[b], h]

Data-parallel over the batch: samples sorted by category on the host and
dealt to cores in contiguous groups of 8, so each core sees 3-4 distinct
categories. All matmul data moves in fp16 (inputs are ~N(0,1) and
0.02*N(0,1); fp16 keeps rel err ~4e-4, far under the 2e-2 gate), which
halves HBM traffic vs fp32.

Per-sample weight matrices are deduplicated at runtime: host passes a
"category changed" flag per sample; W[cat] is DMA'd through a depth-2
predicated TileQueue (dma_start cond=flag skips the transfer but still
bumps the semaphore), prefetched one sample ahead so the next group's
4 MiB load hides under the current sample's ~13 us of matmuls.

Layouts are chosen so every DMA is contiguous per partition:
  x packed on host as  (p, s, kk, t) = x[s, t, p*KK + kk]   -> [128, S*KK*T]
  W fp16 rows i=(p,kk) -> partition p gets rows p*KK..p*KK+KK-1 (32 KB)
  out written as [S*T, H] fp16, upcast to fp32 on the host.

Matmuls: lhsT = x chunk [128(K) x 128(M)], moving rhs = W [128(K) x 1024(N)]
fp16 (full PE rate, 1 row/cycle), accumulating fp32 in PSUM over 8 K-chunks.
"""

import os
import sys

import numpy as np

for _p in (
    "/opt/trn_rl_repo",
    os.path.expanduser("~/.axon_site/_ro/trn_rl_repo"),
):
    if os.path.isdir(_p) and _p not in sys.path:
        sys.path.insert(0, _p)

import concourse.bass as bass  # noqa: E402
import concourse.mybir as mybir  # noqa: E402
import concourse.tile as tile  # noqa: E402
from concourse import bacc  # noqa: E402
from concourse.bass_utils import run_bass_kernel_spmd  # noqa: E402
from concourse.tile import OrderedSet  # noqa: E402

NCORES = 8
B, T, I, H, NCAT = 64, 256, 1024, 2048, 32
S = B // NCORES  # samples per core
KK = I // 128  # K chunks of 128
MM = T // 128  # m-tiles per sample
NN = H // 512  # moving-operand chunks of 512 (one PSUM bank each)
F32 = mybir.dt.float32
F16 = mybir.dt.float16
ET = mybir.EngineType

_cache: dict = {}


def _build(has_bias: bool):
    nc = bacc.Bacc(
        "TRN2", target_bir_lowering=False, debug=False, num_devices=NCORES
    )
    x_in = nc.dram_tensor("xp", [128, S * KK * T], F16, kind="ExternalInput")
    cats_in = nc.dram_tensor("cats", [1, S], mybir.dt.int32, kind="ExternalInput")
    flags_in = nc.dram_tensor("flags", [1, S], mybir.dt.int32, kind="ExternalInput")
    reps_in = nc.dram_tensor("reps", [1, 1], mybir.dt.int32, kind="ExternalInput")
    W_in = nc.dram_tensor("W", [NCAT, I, H], F16, kind="ExternalInput")
    if has_bias:
        b_in = nc.dram_tensor("b", [NCAT, H], F32, kind="ExternalInput")
    out_o = nc.dram_tensor("out", [S * T, H], F16, kind="ExternalOutput")

    with tile.TileContext(nc) as tc:
        with (
            tc.tile_pool(name="const", bufs=1) as cpool,
            tc.tile_pool(name="wqp", bufs=1) as wqpool,
            tc.tile_pool(name="data", bufs=2) as dpool,
            tc.tile_pool(name="mmps", bufs=8, space="PSUM") as mmpool,
        ):
            cats_sb = cpool.tile([1, S], mybir.dt.int32)
            nc.sync.dma_start(cats_sb[:], cats_in[:])
            flags_sb = cpool.tile([1, S], mybir.dt.int32)
            nc.sync.dma_start(flags_sb[:], flags_in[:])
            reps_sb = cpool.tile([1, 1], mybir.dt.int32)
            nc.sync.dma_start(reps_sb[:], reps_in[:])

            cat_vals = [
                nc.values_load(
                    cats_sb[0:1, s : s + 1],
                    engines=(ET.SP,),
                    min_val=0,
                    max_val=NCAT - 1,
                    skip_runtime_bounds_check=True,
                )
                for s in range(S)
            ]
            flag_vals = [
                nc.values_load(
                    flags_sb[0:1, s : s + 1],
                    engines=(ET.PE, ET.SP, ET.DVE) if has_bias else (ET.PE, ET.SP),
                    min_val=0,
                    max_val=1,
                    skip_runtime_bounds_check=True,
                )
                for s in range(S)
            ]
            reps_val = nc.values_load(
                reps_sb[0:1, 0:1],
                min_val=1,
                max_val=1 << 20,
                skip_runtime_bounds_check=True,
            )

            qengines = OrderedSet(
                [ET.PE, ET.SP, ET.DVE] if has_bias else [ET.PE, ET.SP]
            )
            regs = tc.queue_regs(depth=2, engines=qengines, name="wq")
            wq = wqpool.queue([128, KK * H], F16, regs=regs, name="w")
            bq = (
                wqpool.queue([128, H], F32, regs=regs, name="bb")
                if has_bias
                else None
            )

            with tc.For_i(0, reps_val, 1, staggered_reset=True):
                _emit_body(
                    nc, tc, dpool, mmpool, regs, wq, bq,
                    cat_vals, flag_vals, x_in, W_in,
                    b_in if has_bias else None, out_o, has_bias,
                )

    nc.compile()
    return nc


def _load_group(nc, regs, wq, bq, W_in, b_in, cv, fv, has_bias):
    # Split the 4 MiB gather in half so the first K-chunks land (and matmuls
    # can start) while the second half is still streaming.
    regs.advance(fv)
    HALF = (KK // 2) * H
    for _ in regs.specialize():
        src = (
            W_in[bass.ds(cv, 1), :, :]
            .squeeze(0)
            .rearrange("(p kk) h -> p (kk h)", p=128)
        )
        dst = wq.alloc()
        nc.sync.dma_start(dst[:, 0:HALF], src[:, 0:HALF], cond=fv)
        nc.sync.dma_start(dst[:, HALF:], src[:, HALF:], cond=fv)
        if has_bias:
            nc.sync.dma_start(
                bq.alloc(),
                b_in[bass.ds(cv, 1), :].to_broadcast((128, H)),
                cond=fv,
            )


def _emit_body(
    nc, tc, dpool, mmpool, regs, wq, bq,
    cat_vals, flag_vals, x_in, W_in, b_in, out_o, has_bias,
):
    # x per-sample on the scalar (Act) HWDGE ring so the sync ring carries
    # only W gathers; sample 0's x lands first so matmuls start early.
    xt = dpool.tile([128, S * KK * T], F16, tag="xt", bufs=2)
    CH = KK * T  # columns per sample
    nc.scalar.dma_start(xt[:, 0:CH], x_in[:, 0:CH])

    # first group's W (flag[0] is always 1 from the host)
    _load_group(nc, regs, wq, bq, W_in, b_in, cat_vals[0], flag_vals[0], has_bias)

    for s in range(1, S):
        nc.scalar.dma_start(
            xt[:, s * CH : (s + 1) * CH], x_in[:, s * CH : (s + 1) * CH]
        )

    for s in range(S):
        ps = [
            [
                mmpool.tile(
                    [128, 512], F32, tag="ps", bufs=8, name=f"ps_{s}_{th}_{n}"
                )
                for n in range(NN)
            ]
            for th in range(MM)
        ]
        ot = [
            dpool.tile([128, H], F16, tag="ot", bufs=2, name=f"ot_{s}_{th}")
            for th in range(MM)
        ]
        # one specialize per sample: both m-tiles' matmuls in a single
        # 2-arm switch to keep PE branch overhead low
        for _ in regs.specialize():
            w_sb = wq.slot()
            for th in range(MM):
                for kk in range(KK):
                    lhs = xt[:, (s * KK + kk) * T + th * 128 :][:, :128]
                    for n in range(NN):
                        nc.tensor.matmul(
                            ps[th][n][:],
                            lhs,
                            w_sb[:, kk * H + n * 512 :][:, :512],
                            start=(kk == 0),
                            stop=(kk == KK - 1),
                        )
            if has_bias:
                for th in range(MM):
                    for n in range(NN):
                        nc.vector.tensor_add(
                            ot[th][:, n * 512 : (n + 1) * 512],
                            ps[th][n][:],
                            bq.slot()[:, n * 512 : (n + 1) * 512],
                        )
        if not has_bias:
            # drains split between DVE and Act so neither becomes the tail
            for th in range(MM):
                for n in range(NN):
                    if n < NN // 2:
                        nc.vector.tensor_copy(
                            ot[th][:, n * 512 : (n + 1) * 512], ps[th][n][:]
                        )
                    else:
                        nc.scalar.copy(
                            ot[th][:, n * 512 : (n + 1) * 512], ps[th][n][:]
                        )
        for th in range(MM):
            mi = s * MM + th
            nc.scalar.dma_start(out_o[mi * 128 : (mi + 1) * 128, :], ot[th][:])
        # prefetch next sample's group while this sample computes
        if s + 1 < S:
            _load_group(
                nc, regs, wq, bq, W_in, b_in,
                cat_vals[s + 1], flag_vals[s + 1], has_bias,
            )


def _get_nc(has_bias: bool):
    key = ("nc", has_bias)
    if key not in _cache:
        _cache[key] = _build(has_bias)
    return _cache[key]


def _pack_x(xs):
    # xs: [S, T, I] fp32 -> [128, S*KK*T] fp16 with (p, s, kk, t) = xs[s, t, p*KK+kk]
    a = xs.transpose(2, 0, 1)  # [I, S, T]
    a = a.reshape(128, KK, S, T).transpose(0, 2, 1, 3)  # [128, S, KK, T]
    return np.ascontiguousarray(a.reshape(128, S * KK * T).astype(np.float16))


def _make_in_maps(x, cat_ids, W, b, has_bias, order, reps=1, force_flags=None):
    W16 = np.ascontiguousarray(W.astype(np.float16))
    in_maps = []
    for c in range(NCORES):
        idx = order[c * S : (c + 1) * S]
        cats = cat_ids[idx].astype(np.int32)
        flags = np.ones(S, dtype=np.int32)
        flags[1:] = (cats[1:] != cats[:-1]).astype(np.int32)
        if force_flags is not None:
            flags[:] = force_flags
        m = {
            "xp": _pack_x(x[idx]),
            "cats": cats.reshape(1, S),
            "flags": flags.reshape(1, S),
            "reps": np.full((1, 1), reps, dtype=np.int32),
            "W": W16,
        }
        if has_bias:
            m["b"] = b
        in_maps.append(m)
    return in_maps


def _postprocess_core(res_map):
    return res_map["out"].reshape(S, T, H).astype(np.float32)


def kernel(x, cat_ids, W, b):
    x = np.ascontiguousarray(np.asarray(x, dtype=np.float32))
    cat_ids = np.asarray(cat_ids, dtype=np.int32)
    W = np.ascontiguousarray(np.asarray(W, dtype=np.float32))
    b = np.ascontiguousarray(np.asarray(b, dtype=np.float32))
    assert x.shape == (B, T, I) and cat_ids.shape == (B,)
    assert W.shape == (NCAT, I, H) and b.shape == (NCAT, H)

    has_bias = bool(np.any(b))
    nc = _get_nc(has_bias)

    order = np.argsort(cat_ids, kind="stable")
    in_maps = _make_in_maps(x, cat_ids, W, b, has_bias, order)

    res = run_bass_kernel_spmd(nc, in_maps, list(range(NCORES)))

    out = np.empty((B, T, H), dtype=np.float32)
    for c in range(NCORES):
        idx = order[c * S : (c + 1) * S]
        out[idx] = _postprocess_core(res.results[c])
    return out
